# revision 15
# baseline (speedup 1.0000x reference)
"""Trainium2 Bass kernel for CausalModulatedAttention.

Full-input contract: kernel(**inputs) takes the unsharded numpy inputs and
returns the full (B, L, D) float32 output.

Sharding: core = 2*b + g (b = batch, g = row-group).  Row-chunk pairing
exploits causality while keeping one SPMD program: each core owns two
128-row chunks of batch b -- a "lo" chunk needing only j < 256 and a "hi"
chunk needing j < 512:
    g=0: rows [0,128)   (jext 256)  +  rows [384,512) (jext 512)
    g=1: rows [128,256) (jext 256)  +  rows [256,384) (jext 512)

Per core:
  - projections q/k/v in transposed layouts straight from x^T (PE)
  - pairwise causal-graph bias G: gelu(he[j,c]+hc[i,c]+b1[c]) as one ACT op
    per 4-row group (per-partition bias), reduced over c via per-t
    stationary matrices on PE
  - scores = q.k^T (PE); bias+mask tile added into PSUM on DVE
  - softmax without max-subtraction; Exp emits row sums via accum_out
  - transpose+normalize fused: EnT = E_chunk^T @ diag(1/rowsum) on PE
  - attn @ v on PE (col-group packed head pairs); output proj on PE
All matmul operands bf16, fp32 PSUM accumulation.  Inputs arrive as a few
large packed arrays to minimize DMA issue overhead; projection matmuls are
emitted interleaved with attention to keep the PE array warm.
"""

import math

import numpy as np
import ml_dtypes

import concourse.bass as bass
import concourse.mybir as mybir
import concourse.tile as tile
from concourse import bacc
from concourse.bass_utils import run_bass_kernel_spmd

BF = mybir.dt.bfloat16
F32 = mybir.dt.float32
AF = mybir.ActivationFunctionType
ALU = mybir.AluOpType

B, L, D = 4, 512, 1024
H, HD, CD = 16, 64, 32
ALPHA = 0.3
N_CORES = 8
RPC = 256             # rows per core
JEXT = [256, 512]     # score width for (lo, hi) row chunk
NEG = -1.0e30
MW = JEXT[0] + JEXT[1]
# packed bf16 consts: w2t (4096) | mask (768) | ident (128)
CPK = 4096 + MW + 128


def _bf(a):
    return np.ascontiguousarray(a.astype(ml_dtypes.bfloat16))


def _f32(a):
    return np.ascontiguousarray(a.astype(np.float32))


def core_rows(g):
    """Global row ranges (lo, hi) owned by row-group g."""
    lo = range(g * 128, g * 128 + 128)
    hi = range(384 - g * 128, 384 - g * 128 + 128)
    return lo, hi


def build_program():
    nc = bacc.Bacc("TRN2", target_bir_lowering=False, debug=False)

    boot_d = nc.dram_tensor("boot", [128, 512], BF, kind="ExternalInput")
    xall_d = nc.dram_tensor("xall", [128, 8 * L + 8 * RPC], BF, kind="ExternalInput")
    cpk_d = nc.dram_tensor("cpk", [128, CPK], BF, kind="ExternalInput")
    wk_d = nc.dram_tensor("wka", [128, 8 * D], BF, kind="ExternalInput")
    wq_d = nc.dram_tensor("wqa", [128, 8 * D], BF, kind="ExternalInput")
    wv_d = nc.dram_tensor("wva", [128, 8 * D], BF, kind="ExternalInput")
    wo_d = nc.dram_tensor("woa", [128, 8 * D], BF, kind="ExternalInput")
    b1_d = nc.dram_tensor("b1c", [CD, 1], F32, kind="ExternalInput")
    b2_d = nc.dram_tensor("b2h", [128, 1], F32, kind="ExternalInput")
    out_d = nc.dram_tensor("out", [RPC, D], F32, kind="ExternalOutput")

    with tile.TileContext(nc) as tc:
        with (
            tc.tile_pool(name="consts", bufs=1) as consts,
            tc.tile_pool(name="work", bufs=3) as work,
            tc.tile_pool(name="entp", bufs=6) as entp,
            tc.tile_pool(name="ppbig", bufs=3, space="PSUM") as ppbig,
            tc.tile_pool(name="ppg", bufs=1, space="PSUM") as ppg,
            tc.tile_pool(name="pptp", bufs=2, space="PSUM") as pptp,
            tc.tile_pool(name="ppot", bufs=2, space="PSUM") as ppot,
        ):
            def load(name, shape, dt, src):
                t = consts.tile(shape, dt, tag=name)
                nc.sync.dma_start(out=t[:], in_=src)
                return t

            boot = load("boot", [128, 512], BF, boot_d[:, :])
            xall = load("xall", [128, 8 * L + 8 * RPC], BF, xall_d[:, :])
            cpk = load("cpk", [128, CPK], BF, cpk_d[:, :])
            b1c = load("b1c", [CD, 1], F32, b1_d[:, :])
            b2h = load("b2h", [128, 1], F32, b2_d[:, :])
            wka = load("wka", [128, 8 * D], BF, wk_d[:, :])
            wqa = load("wqa", [128, 8 * D], BF, wq_d[:, :])
            wva = load("wva", [128, 8 * D], BF, wv_d[:, :])
            woa = load("woa", [128, 8 * D], BF, wo_d[:, :])

            xT = [xall[:, mc * L:(mc + 1) * L] for mc in range(8)]
            xTq = [xall[:, 8 * L + mc * RPC: 8 * L + (mc + 1) * RPC]
                   for mc in range(8)]
            wc1 = boot[:, 0:256]
            we1 = boot[:, 256:512]
            w2t = cpk[:, 0:4096]
            maskc = cpk[:, 4096:4096 + MW]
            ident = cpk[:, 4096 + MW:4096 + MW + 128]
            wk = [wka[:, mc * D:(mc + 1) * D] for mc in range(8)]
            wq = [wqa[:, mc * D:(mc + 1) * D] for mc in range(8)]
            wv = [wva[:, mc * D:(mc + 1) * D] for mc in range(8)]
            wo = [woa[:, mc * D:(mc + 1) * D] for mc in range(8)]

            # ---------- small projections first (unblock the gelu chain) ----
            ps = ppbig.tile([CD, L], F32, tag="ps")
            for mc in range(8):
                nc.tensor.matmul(ps[:], we1[:, mc * CD:(mc + 1) * CD], xT[mc],
                                 start=(mc == 0), stop=(mc == 7))
            he4 = consts.tile([128, L], BF, tag="he4")
            nc.vector.tensor_copy(he4[0:CD, :], ps[:])
            for u in range(1, 4):
                nc.sync.dma_start(out=he4[u * CD:(u + 1) * CD, :], in_=he4[0:CD, :])

            ps = ppbig.tile([CD, RPC], F32, tag="ps")
            for mc in range(8):
                nc.tensor.matmul(ps[:], wc1[:, mc * CD:(mc + 1) * CD], xTq[mc],
                                 start=(mc == 0), stop=(mc == 7))
            hcbT = consts.tile([CD, RPC], F32, tag="hcbT")
            nc.vector.tensor_scalar_add(hcbT[:], ps[:], b1c[:, 0:1])
            hc4 = consts.tile([128, 64], F32, tag="hc4")
            hsrc = hcbT[:, :].rearrange("p (a t f) -> p a t f", a=2, t=32)
            for u in range(4):
                nc.sync.dma_start(
                    out=hc4[u * CD:(u + 1) * CD, :].rearrange("p (a t) -> p a t", a=2),
                    in_=hsrc[:, :, :, u])

            # ---------- pairwise causal-graph bias ----------
            gbm = [None, None]

            def pairwise(ic):
                jx = JEXT[ic]
                graw = ppg.tile([128, 512], F32, tag="graw")
                for t in range(32):
                    ga = work.tile([128, jx], BF, tag=f"ga{ic}")
                    nc.scalar.activation(ga[:], he4[:, :jx], AF.Gelu,
                                         bias=hc4[:, ic * 32 + t: ic * 32 + t + 1])
                    nc.tensor.matmul(graw[:, :jx], w2t[:, t * 128:(t + 1) * 128],
                                     ga[:], start=(t == 0), stop=(t == 31))
                th = work.tile([128, jx], BF, tag=f"th{ic}")
                nc.scalar.activation(th[:], graw[:, :jx], AF.Tanh, scale=0.5,
                                     bias=b2h[:, 0:1])
                g = consts.tile([128, jx], BF, tag=f"gbm{ic}")
                moff = 0 if ic == 0 else JEXT[0]
                nc.vector.scalar_tensor_tensor(
                    g[:], th[:], ALPHA / 2.0, maskc[:, moff:moff + jx],
                    op0=ALU.mult, op1=ALU.add)
                gbm[ic] = g

            # ---------- projection emitters ----------
            kT, qT, v = [None] * 8, [None] * 8, [None] * 8

            def proj_kq(dc):
                ps = ppbig.tile([128, L], F32, tag="ps")
                for mc in range(8):
                    nc.tensor.matmul(ps[:], wk[mc][:, dc * 128:(dc + 1) * 128],
                                     xT[mc], start=(mc == 0), stop=(mc == 7))
                t = consts.tile([128, L], BF, tag=f"kT{dc}")
                nc.vector.tensor_copy(t[:], ps[:])
                kT[dc] = t
                ps = ppbig.tile([128, RPC], F32, tag="ps")
                for mc in range(8):
                    nc.tensor.matmul(ps[:], wq[mc][:, dc * 128:(dc + 1) * 128],
                                     xTq[mc], start=(mc == 0), stop=(mc == 7))
                t = consts.tile([128, RPC], BF, tag=f"qT{dc}")
                nc.vector.tensor_copy(t[:], ps[:])
                qT[dc] = t

            def proj_v(jc):
                t = consts.tile([128, D], BF, tag=f"v{jc}")
                for nn in range(2):
                    ps = ppbig.tile([128, 512], F32, tag="ps")
                    for mc in range(8):
                        nc.tensor.matmul(ps[:], xT[mc][:, jc * 128:(jc + 1) * 128],
                                         wv[mc][:, nn * 512:(nn + 1) * 512],
                                         start=(mc == 0), stop=(mc == 7))
                    nc.vector.tensor_copy(t[:, nn * 512:(nn + 1) * 512], ps[:])
                v[jc] = t

            # ---------- attention ----------
            ot = [[None] * 8, [None] * 8]

            def attention(ic, hp):
                jx = JEXT[ic]
                njc = jx // 128
                otp = ppot.tile([128, 128], F32, tag="otp")
                for sub in range(2):
                    h = 2 * hp + sub
                    po = 64 * sub
                    sc = ppbig.tile([128, 512], F32, tag="ps")
                    nc.tensor.matmul(
                        sc[:, :jx], qT[hp][po:po + 64, ic * 128:(ic + 1) * 128],
                        kT[hp][po:po + 64, :jx], start=True, stop=True,
                        tile_position=(po, 0))
                    nc.vector.tensor_add(sc[:, :jx], sc[:, :jx], gbm[ic][:])
                    e = work.tile([128, jx], BF, tag=f"e{ic}")
                    sums = work.tile([128, 1], F32, tag="sums")
                    nc.scalar.activation(e[:], sc[:, :jx], AF.Exp,
                                         accum_out=sums[:, 0:1])
                    inv = work.tile([128, 1], F32, tag="inv")
                    nc.vector.reciprocal(inv[:], sums[:])
                    dg = work.tile([128, 128], BF, tag="dg")
                    nc.vector.tensor_scalar_mul(dg[:], ident, inv[:, 0:1])
                    for jc in range(njc):
                        etp = pptp.tile([128, 128], F32, tag="etp")
                        nc.tensor.matmul(etp[:], e[:, jc * 128:(jc + 1) * 128],
                                         dg[:], start=True, stop=True)
                        ent = entp.tile([128, 128], BF, tag="ent")
                        if jc % 2 == 0:
                            nc.vector.tensor_copy(ent[:], etp[:])
                        else:
                            nc.scalar.copy(ent[:], etp[:])
                        nc.tensor.matmul(
                            otp[po:po + 64, :], v[jc][:, h * HD:(h + 1) * HD],
                            ent[:], start=(jc == 0), stop=(jc == njc - 1),
                            tile_position=(0, po))
                t = consts.tile([128, 128], BF, tag=f"ot{ic}_{hp}")
                nc.vector.tensor_copy(t[:], otp[:])
                ot[ic][hp] = t

            def out_proj(ic, nn):
                ps = ppbig.tile([128, 512], F32, tag="ps")
                for dc in range(8):
                    nc.tensor.matmul(ps[:], ot[ic][dc][:],
                                     wo[dc][:, nn * 512:(nn + 1) * 512],
                                     start=(dc == 0), stop=(dc == 7))
                osb = work.tile([128, 512], F32, tag="osb")
                nc.vector.tensor_copy(osb[:], ps[:])
                nc.sync.dma_start(
                    out=out_d[ic * 128:(ic + 1) * 128, nn * 512:(nn + 1) * 512],
                    in_=osb[:])

            # emission order: all gelu-phase ACT work first (one table-set
            # switch total), then attention(ic0) interleaved with the k/q/v
            # projections, then attention(ic1) interleaved with the deferred
            # v[2], v[3] and ic0 output projections as PE warm-keepers.
            pairwise(0)
            pairwise(1)
            for hp in range(8):
                proj_kq(hp)
                if hp == 0:
                    proj_v(0)
                    proj_v(1)
                attention(0, hp)
            proj_v(2)
            proj_v(3)
            for hp in range(8):
                attention(1, hp)
                if hp in (1, 4):
                    out_proj(0, 0 if hp == 1 else 1)
            out_proj(1, 0)
            out_proj(1, 1)

    nc.compile()
    return nc


def _host_inputs(x, Wq, Wk, Wv, Wo, Wc, We, W1c, W1e, b1, W2, b2):
    """Per-core input dicts (host-side shard/cast/pack)."""
    x = _f32(np.asarray(x))
    wq_s = _f32(np.asarray(Wq) / math.sqrt(HD))
    wk = _f32(np.asarray(Wk))
    wv = _f32(np.asarray(Wv))
    wo = _f32(np.asarray(Wo))
    wc1 = _f32(np.asarray(Wc) @ np.asarray(W1c))      # (D, CD)
    we1 = _f32(np.asarray(We) @ np.asarray(W1e))
    wc1r = wc1.reshape(8, 128, CD).transpose(1, 0, 2).reshape(128, 8 * CD)
    we1r = we1.reshape(8, 128, CD).transpose(1, 0, 2).reshape(128, 8 * CD)
    b1c = _f32(np.asarray(b1).reshape(CD, 1))
    b2h = _f32(np.full((128, 1), 0.5 * float(np.asarray(b2).reshape(-1)[0])))
    w2 = _f32(np.asarray(W2))

    def hpack(w):  # (1024, 1024) -> (128, 8*1024) chunk-major
        return w.reshape(8, 128, D).transpose(1, 0, 2).reshape(128, 8 * D)

    wka, wqa, wva, woa = (_bf(hpack(w)) for w in (wk, wq_s, wv, wo))

    # w2t[p=u*32+c, t*128 + m] = W2[c] if m == 4t+u else 0
    w2t = np.zeros((32, 128, 128), np.float32)
    for t in range(32):
        for u in range(4):
            w2t[t, u * CD:(u + 1) * CD, 4 * t + u] = w2
    w2t = w2t.transpose(1, 0, 2).reshape(128, 32 * 128)

    identb = np.eye(128, dtype=np.float32)

    in_maps = []
    for core in range(N_CORES):
        b, g = core // 2, core % 2
        lo, hi = core_rows(g)
        rows = np.concatenate([np.arange(lo.start, lo.stop),
                               np.arange(hi.start, hi.stop)])
        xTb = np.ascontiguousarray(x[b].T)            # (D, L)
        mask = np.zeros((128, MW), np.float32)
        moff = 0
        for ic, rng in enumerate((lo, hi)):
            jx = JEXT[ic]
            jj = np.arange(jx)[None, :]
            rr = np.arange(rng.start, rng.stop)[:, None]
            mask[:, moff:moff + jx] = np.where(jj <= rr, 0.0, NEG)
            moff += jx
        xTb8 = xTb.reshape(8, 128, L).transpose(1, 0, 2).reshape(128, 8 * L)
        xTq8 = (xTb[:, rows].reshape(8, 128, RPC)
                .transpose(1, 0, 2).reshape(128, 8 * RPC))
        xallc = np.concatenate([xTb8, xTq8], axis=1)
        bootc = np.concatenate([wc1r, we1r], axis=1)
        cpkc = np.concatenate([w2t, mask, identb], axis=1)
        in_maps.append({
            "boot": _bf(bootc), "xall": _bf(xallc), "cpk": _bf(cpkc),
            "wka": wka, "wqa": wqa, "wva": wva, "woa": woa,
            "b1c": b1c, "b2h": b2h,
        })
    return in_maps


def run(inputs: dict, trace: bool = False):
    """Build, run on 8 cores, return (full_output, BassKernelResults)."""
    nc = build_program()
    in_maps = _host_inputs(**inputs)
    res = run_bass_kernel_spmd(nc, in_maps, core_ids=list(range(N_CORES)),
                               trace=trace)
    out = np.zeros((B, L, D), np.float32)
    for core in range(N_CORES):
        b, g = core // 2, core % 2
        lo, hi = core_rows(g)
        out[b, lo.start:lo.stop, :] = res.results[core]["out"][0:128]
        out[b, hi.start:hi.stop, :] = res.results[core]["out"][128:256]
    return out, res


def kernel(**inputs) -> np.ndarray:
    out, _ = run(inputs, trace=False)
    return out
